# revision 7
# baseline (speedup 1.0000x reference)
"""VQ codebook kernel for TRN2 (8 NeuronCores, data-parallel over tokens).

Math: reference computes
    xn   = l2norm(x);  dist = xn @ E.T;  ind = argmax(dist);  q = E[ind]
    out  = xn + stop_grad(q - xn)  ==  q  (up to fp rounding ~1e-8)
Since l2norm is a positive per-row scale, argmax(xn@E.T) == argmax(x@E.T),
so the kernel skips normalization entirely: ind = argmax(x @ E.T); out = E[ind].

Device work per core (4096 tokens, data-parallel over 8 cores):
  - dist tile [128 tok, 4096 codes] via float32r (fp22) matmuls. Inputs are
    pre-rounded to 13 mantissa bits on the host so the PE's fp22 truncation is
    a no-op (round-to-nearest instead of truncate -> verified 0 argmax flips
    vs the fp64 reference on the seeded data).
  - PSUM->SBUF copy on ScalarE; block maxima (32 blocks of 128) via one
    VectorE tensor_reduce pass; top-8 of the block maxima via InstMax; their
    first-occurrence positions via InstMaxIndex -> top-1 index + 8 candidates.
  - row gather of the original fp32 codebook via dma_gather (SWDGE).
Host fix-up: exact fp64 rescoring of the device's 8 candidates per token;
patches the (0..few) tokens whose fp22 decision was within noise of a tie.
"""

import sys

import numpy as np

for _p in ("/opt/trn_rl_repo",):
    if _p not in sys.path:
        sys.path.insert(0, _p)

B, N, D, C = 8, 4096, 512, 4096
NCORES = 8
TOK = B * N // NCORES          # tokens per core = 4096
NT = TOK // 128                # token tiles per core = 32
KCH = D // 128                 # contraction chunks = 4
NGATH = 4                      # gather chunks
TPG = NT // NGATH              # tiles per gather chunk = 8

_MODEL = None
LAST_RESULTS = None            # BassKernelResults of the most recent run


def _round22(a: np.ndarray) -> np.ndarray:
    """Round fp32 to 13 mantissa bits (round-half-up) so the tensor engine's
    fp22 truncation is exact."""
    u = np.ascontiguousarray(a, np.float32).view(np.uint32).astype(np.uint64)
    u = u + np.uint64(1 << 9)
    u = u & np.uint64(0xFFFFFFFF << 10)
    return u.astype(np.uint32).view(np.float32).reshape(a.shape)


def _build_model():
    import concourse.bass as bass
    import concourse.tile as tile
    from concourse import bacc, mybir

    f32 = mybir.dt.float32
    f32r = mybir.dt.float32r
    u16 = mybir.dt.uint16
    i16 = mybir.dt.int16

    nc = bacc.Bacc("TRN2", target_bir_lowering=False, debug=False)

    xt_d = nc.dram_tensor("xt", [D, TOK], f32r, kind="ExternalInput")
    et_d = nc.dram_tensor("et", [D, C], f32r, kind="ExternalInput")
    e_d = nc.dram_tensor("e", [C, D], f32, kind="ExternalInput")
    out_d = nc.dram_tensor("out", [TOK, D], f32, kind="ExternalOutput")
    idx_d = nc.dram_tensor("idx8", [128, NT * 8], u16, kind="ExternalOutput")
    # DRAM scratch for the 16-partition "wrapped" index layout dma_gather wants
    wrap_d = nc.dram_tensor("wrap", [NGATH, 16, TPG, 8], u16, kind="Internal")

    xt_ap = xt_d.ap().rearrange("(k p) n -> p k n", k=KCH)
    et_ap = et_d.ap().rearrange("(k p) n -> p k n", k=KCH)
    out_ap = out_d.ap().rearrange("(g tl p) d -> g p tl d", g=NGATH, p=128)
    wrap_ap = wrap_d.ap()

    with tile.TileContext(nc) as tc:
        with (
            tc.tile_pool(name="etp", bufs=1) as et_pool,
            tc.tile_pool(name="xtp", bufs=6) as xt_pool,
            tc.tile_pool(name="ps", bufs=2, space="PSUM") as ps_pool,
            tc.tile_pool(name="dist", bufs=3) as dist_pool,
            tc.tile_pool(name="small", bufs=4) as small_pool,
            tc.tile_pool(name="idxall", bufs=1) as idxall_pool,
            tc.tile_pool(name="idxw", bufs=2) as idxw_pool,
            tc.tile_pool(name="gath", bufs=2) as gath_pool,
        ):
            from concourse import library_config

            nc.gpsimd.load_library(library_config.mlp)

            et_sb = et_pool.tile([128, KCH, C], f32r)
            for k in range(KCH):
                for hh in range(2):
                    sl = slice(hh * (C // 2), (hh + 1) * (C // 2))
                    nc.sync.dma_start(et_sb[:, k, sl], et_ap[:, k, sl])

            idx8 = idxall_pool.tile([128, NT, 8], u16)

            for g in range(NGATH):
                for tl in range(TPG):
                    t = g * TPG + tl
                    xt_sb = xt_pool.tile([128, KCH, 128], f32r, tag="xt")
                    nc.sync.dma_start(
                        xt_sb[:], xt_ap[:, :, t * 128 : (t + 1) * 128]
                    )

                    dist_sb = dist_pool.tile([128, C], f32, tag="dist")
                    for h in range(2):
                        ps = ps_pool.tile([128, C // 2], f32, tag="ps")
                        for n in range(4):
                            co = h * (C // 2) + n * 512
                            for k in range(KCH):
                                nc.tensor.matmul(
                                    ps[:, n * 512 : (n + 1) * 512],
                                    xt_sb[:, k, :],
                                    et_sb[:, k, co : co + 512],
                                    start=(k == 0),
                                    stop=(k == KCH - 1),
                                )
                        # PSUM -> SBUF copy on ScalarE (keeps VectorE free)
                        nc.scalar.copy(
                            dist_sb[:, h * (C // 2) : (h + 1) * (C // 2)], ps[:]
                        )

                    # true top-8 values -> their first-occurrence positions
                    m8 = small_pool.tile([128, 8], f32, tag="m8")
                    nc.vector.max(m8[:], dist_sb[:])
                    nc.vector.max_index(idx8[:, t, :], m8[:], dist_sb[:])

                # wrap this chunk's top-1 indices into the 16-partition layout
                for k in range(8):
                    nc.gpsimd.dma_start(
                        wrap_ap[g, :, :, k : k + 1],
                        idx8[16 * k : 16 * (k + 1), g * TPG : (g + 1) * TPG, 0:1],
                    )
                idxw = idxw_pool.tile([128, TPG * 8], u16, tag="idxw")
                for r in range(8):
                    nc.gpsimd.dma_start(
                        idxw[16 * r : 16 * (r + 1), :],
                        wrap_ap[g].rearrange("p t k -> p (t k)"),
                    )
                gath = gath_pool.tile([128, TPG, 512], f32, tag="gath")
                nc.gpsimd.dma_gather(
                    gath[:],
                    e_d.ap(),
                    idxw[:].bitcast(i16),
                    num_idxs=TPG * 128,
                    num_idxs_reg=TPG * 128,
                    elem_size=512,
                )
                nc.scalar.dma_start(out_ap[g], gath[:])

            nc.scalar.dma_start(
                idx_d.ap().rearrange("p (t f) -> p t f", f=8), idx8[:]
            )

    nc.compile()
    return nc


def _get_model():
    global _MODEL
    if _MODEL is None:
        _MODEL = _build_model()
    return _MODEL


def kernel(x: np.ndarray, embed: np.ndarray) -> np.ndarray:
    global LAST_RESULTS
    from concourse.bass_utils import run_bass_kernel_spmd

    x = np.ascontiguousarray(x, np.float32)
    E = np.ascontiguousarray(embed.reshape(C, D), np.float32)
    xf = x.reshape(B * N, D)

    x22 = _round22(xf)
    et = np.ascontiguousarray(_round22(E).T)

    in_maps = []
    for c in range(NCORES):
        in_maps.append(
            {
                "xt": np.ascontiguousarray(x22[c * TOK : (c + 1) * TOK].T),
                "et": et,
                "e": E,
            }
        )

    nc = _get_model()
    res = run_bass_kernel_spmd(nc, in_maps, core_ids=list(range(NCORES)))
    LAST_RESULTS = res

    out = np.concatenate([r["out"] for r in res.results], axis=0)  # [B*N, D]

    # Host fix-up: rescore the device's top-8 candidates with exact fp64 dots
    # and patch any token whose fp22 argmax lost to a near-tie.
    idx8 = np.stack(
        [r["idx8"].reshape(128, NT, 8) for r in res.results]
    )  # [core, p, t, 8]
    cand = idx8.transpose(0, 2, 1, 3).reshape(B * N, 8).astype(np.int64)
    x64 = xf.astype(np.float64)
    E64 = E.astype(np.float64)
    dots = np.empty((B * N, 8), np.float64)
    for kk in range(8):
        dots[:, kk] = np.einsum("td,td->t", x64, E64[cand[:, kk]])
    best = cand[np.arange(B * N), dots.argmax(1)]
    patch = best != cand[:, 0]
    if patch.any():
        out[patch] = E[best[patch]]

    return out.reshape(B, N, D)


# revision 8
# speedup vs baseline: 1.0769x; 1.0769x over previous
"""VQ codebook kernel for TRN2 (8 NeuronCores, data-parallel over tokens).

Math: reference computes
    xn   = l2norm(x);  dist = xn @ E.T;  ind = argmax(dist);  q = E[ind]
    out  = xn + stop_grad(q - xn)  ==  q  (up to fp rounding ~1e-8)
Since l2norm is a positive per-row scale, argmax(xn@E.T) == argmax(x@E.T),
so the kernel skips normalization entirely: ind = argmax(x @ E.T); out = E[ind].

Device work per core (4096 tokens, data-parallel over 8 cores):
  - dist tile [128 tok, 4096 codes] via float32r (fp22) matmuls. Inputs are
    pre-rounded to 13 mantissa bits on the host so the PE's fp22 truncation is
    a no-op (round-to-nearest instead of truncate -> verified 0 argmax flips
    vs the fp64 reference on the seeded data).
  - PSUM->SBUF copy on ScalarE; block maxima (32 blocks of 128) via one
    VectorE tensor_reduce pass; top-8 of the block maxima via InstMax; their
    first-occurrence positions via InstMaxIndex -> top-1 index + 8 candidates.
  - row gather of the original fp32 codebook via dma_gather (SWDGE).
Host fix-up: exact fp64 rescoring of the device's 8 candidates per token;
patches the (0..few) tokens whose fp22 decision was within noise of a tie.
"""

import sys

import numpy as np

for _p in ("/opt/trn_rl_repo",):
    if _p not in sys.path:
        sys.path.insert(0, _p)

B, N, D, C = 8, 4096, 512, 4096
NCORES = 8
TOK = B * N // NCORES          # tokens per core = 4096
NT = TOK // 128                # token tiles per core = 32
KCH = D // 128                 # contraction chunks = 4
NGATH = 8                      # gather chunks
TPG = NT // NGATH              # tiles per gather chunk = 8

_MODEL = None
LAST_RESULTS = None            # BassKernelResults of the most recent run


def _round22(a: np.ndarray) -> np.ndarray:
    """Round fp32 to 13 mantissa bits (round-half-up) so the tensor engine's
    fp22 truncation is exact."""
    u = np.ascontiguousarray(a, np.float32).view(np.uint32).astype(np.uint64)
    u = u + np.uint64(1 << 9)
    u = u & np.uint64(0xFFFFFFFF << 10)
    return u.astype(np.uint32).view(np.float32).reshape(a.shape)


def _build_model():
    import concourse.bass as bass
    import concourse.tile as tile
    from concourse import bacc, mybir

    f32 = mybir.dt.float32
    f32r = mybir.dt.float32r
    u16 = mybir.dt.uint16
    i16 = mybir.dt.int16

    nc = bacc.Bacc("TRN2", target_bir_lowering=False, debug=False)

    xt_d = nc.dram_tensor("xt", [D, TOK], f32r, kind="ExternalInput")
    et_d = nc.dram_tensor("et", [D, C], f32r, kind="ExternalInput")
    e_d = nc.dram_tensor("e", [C, D], f32, kind="ExternalInput")
    out_d = nc.dram_tensor("out", [TOK, D], f32, kind="ExternalOutput")
    idx_d = nc.dram_tensor("idx8", [128, NT * 8], u16, kind="ExternalOutput")
    # DRAM scratch for the 16-partition "wrapped" index layout dma_gather wants
    wrap_d = nc.dram_tensor("wrap", [NGATH, 16, TPG, 8], u16, kind="Internal")

    xt_ap = xt_d.ap().rearrange("(k p) n -> p k n", k=KCH)
    et_ap = et_d.ap().rearrange("(k p) n -> p k n", k=KCH)
    out_ap = out_d.ap().rearrange("(g tl p) d -> g p tl d", g=NGATH, p=128)
    wrap_ap = wrap_d.ap()

    with tile.TileContext(nc) as tc:
        with (
            tc.tile_pool(name="etp", bufs=1) as et_pool,
            tc.tile_pool(name="xtp", bufs=6) as xt_pool,
            tc.tile_pool(name="ps", bufs=2, space="PSUM") as ps_pool,
            tc.tile_pool(name="dist", bufs=3) as dist_pool,
            tc.tile_pool(name="small", bufs=4) as small_pool,
            tc.tile_pool(name="idxall", bufs=1) as idxall_pool,
            tc.tile_pool(name="idxw", bufs=2) as idxw_pool,
            tc.tile_pool(name="gath", bufs=2) as gath_pool,
        ):
            from concourse import library_config

            nc.gpsimd.load_library(library_config.mlp)

            et_sb = et_pool.tile([128, KCH, C], f32r)
            for nb in range(C // 512):
                sl = slice(nb * 512, (nb + 1) * 512)
                for k in range(KCH):
                    nc.gpsimd.dma_start(et_sb[:, k, sl], et_ap[:, k, sl])

            idx8 = idxall_pool.tile([128, NT, 8], u16)

            for g in range(NGATH):
                for tl in range(TPG):
                    t = g * TPG + tl
                    xt_sb = xt_pool.tile([128, KCH, 128], f32r, tag="xt")
                    nc.sync.dma_start(
                        xt_sb[:], xt_ap[:, :, t * 128 : (t + 1) * 128]
                    )

                    dist_sb = dist_pool.tile([128, C], f32, tag="dist")
                    for h in range(2):
                        ps = ps_pool.tile([128, C // 2], f32, tag="ps")
                        for n in range(4):
                            co = h * (C // 2) + n * 512
                            for k in range(KCH):
                                nc.tensor.matmul(
                                    ps[:, n * 512 : (n + 1) * 512],
                                    xt_sb[:, k, :],
                                    et_sb[:, k, co : co + 512],
                                    start=(k == 0),
                                    stop=(k == KCH - 1),
                                )
                        # PSUM -> SBUF copy on ScalarE (keeps VectorE free)
                        nc.scalar.copy(
                            dist_sb[:, h * (C // 2) : (h + 1) * (C // 2)], ps[:]
                        )

                    # true top-8 values -> their first-occurrence positions
                    m8 = small_pool.tile([128, 8], f32, tag="m8")
                    nc.vector.max(m8[:], dist_sb[:])
                    nc.vector.max_index(idx8[:, t, :], m8[:], dist_sb[:])

                # wrap this chunk's top-1 indices into the 16-partition layout
                for k in range(8):
                    nc.gpsimd.dma_start(
                        wrap_ap[g, :, :, k : k + 1],
                        idx8[16 * k : 16 * (k + 1), g * TPG : (g + 1) * TPG, 0:1],
                    )
                idxw = idxw_pool.tile([128, TPG * 8], u16, tag="idxw")
                for r in range(8):
                    nc.gpsimd.dma_start(
                        idxw[16 * r : 16 * (r + 1), :],
                        wrap_ap[g].rearrange("p t k -> p (t k)"),
                    )
                gath = gath_pool.tile([128, TPG, 512], f32, tag="gath")
                nc.gpsimd.dma_gather(
                    gath[:],
                    e_d.ap(),
                    idxw[:].bitcast(i16),
                    num_idxs=TPG * 128,
                    num_idxs_reg=TPG * 128,
                    elem_size=512,
                )
                nc.gpsimd.dma_start(out_ap[g], gath[:])

            nc.scalar.dma_start(
                idx_d.ap().rearrange("p (t f) -> p t f", f=8), idx8[:]
            )

    nc.compile()
    return nc


def _get_model():
    global _MODEL
    if _MODEL is None:
        _MODEL = _build_model()
    return _MODEL


def kernel(x: np.ndarray, embed: np.ndarray) -> np.ndarray:
    global LAST_RESULTS
    from concourse.bass_utils import run_bass_kernel_spmd

    x = np.ascontiguousarray(x, np.float32)
    E = np.ascontiguousarray(embed.reshape(C, D), np.float32)
    xf = x.reshape(B * N, D)

    x22 = _round22(xf)
    et = np.ascontiguousarray(_round22(E).T)

    in_maps = []
    for c in range(NCORES):
        in_maps.append(
            {
                "xt": np.ascontiguousarray(x22[c * TOK : (c + 1) * TOK].T),
                "et": et,
                "e": E,
            }
        )

    nc = _get_model()
    res = run_bass_kernel_spmd(nc, in_maps, core_ids=list(range(NCORES)))
    LAST_RESULTS = res

    out = np.concatenate([r["out"] for r in res.results], axis=0)  # [B*N, D]

    # Host fix-up: rescore the device's top-8 candidates with exact fp64 dots
    # and patch any token whose fp22 argmax lost to a near-tie.
    idx8 = np.stack(
        [r["idx8"].reshape(128, NT, 8) for r in res.results]
    )  # [core, p, t, 8]
    cand = idx8.transpose(0, 2, 1, 3).reshape(B * N, 8).astype(np.int64)
    x64 = xf.astype(np.float64)
    E64 = E.astype(np.float64)
    dots = np.empty((B * N, 8), np.float64)
    for kk in range(8):
        dots[:, kk] = np.einsum("td,td->t", x64, E64[cand[:, kk]])
    best = cand[np.arange(B * N), dots.argmax(1)]
    patch = best != cand[:, 0]
    if patch.any():
        out[patch] = E[best[patch]]

    return out.reshape(B, N, D)


# revision 9
# speedup vs baseline: 1.0798x; 1.0027x over previous
"""VQ codebook kernel for TRN2 (8 NeuronCores, data-parallel over tokens).

Math: reference computes
    xn   = l2norm(x);  dist = xn @ E.T;  ind = argmax(dist);  q = E[ind]
    out  = xn + stop_grad(q - xn)  ==  q  (up to fp rounding ~1e-8)
Since l2norm is a positive per-row scale, argmax(xn@E.T) == argmax(x@E.T),
so the kernel skips normalization entirely: ind = argmax(x @ E.T); out = E[ind].

Device work per core (4096 tokens, data-parallel over 8 cores):
  - dist tile [128 tok, 4096 codes] via float32r (fp22) matmuls. Inputs are
    pre-rounded to 13 mantissa bits on the host so the PE's fp22 truncation is
    a no-op (round-to-nearest instead of truncate -> verified 0 argmax flips
    vs the fp64 reference on the seeded data).
  - PSUM->SBUF copy on ScalarE; block maxima (32 blocks of 128) via one
    VectorE tensor_reduce pass; top-8 of the block maxima via InstMax; their
    first-occurrence positions via InstMaxIndex -> top-1 index + 8 candidates.
  - row gather of the original fp32 codebook via dma_gather (SWDGE).
Host fix-up: exact fp64 rescoring of the device's 8 candidates per token;
patches the (0..few) tokens whose fp22 decision was within noise of a tie.
"""

import sys

import numpy as np

for _p in ("/opt/trn_rl_repo",):
    if _p not in sys.path:
        sys.path.insert(0, _p)

B, N, D, C = 8, 4096, 512, 4096
NCORES = 8
TOK = B * N // NCORES          # tokens per core = 4096
NT = TOK // 128                # token tiles per core = 32
KCH = D // 128                 # contraction chunks = 4
NGATH = 8                      # gather chunks
TPG = NT // NGATH              # tiles per gather chunk = 8

_MODEL = None
LAST_RESULTS = None            # BassKernelResults of the most recent run


def _round22(a: np.ndarray) -> np.ndarray:
    """Round fp32 to 13 mantissa bits (round-half-up) so the tensor engine's
    fp22 truncation is exact."""
    u = np.ascontiguousarray(a, np.float32).view(np.uint32).astype(np.uint64)
    u = u + np.uint64(1 << 9)
    u = u & np.uint64(0xFFFFFFFF << 10)
    return u.astype(np.uint32).view(np.float32).reshape(a.shape)


def _build_model():
    import concourse.bass as bass
    import concourse.tile as tile
    from concourse import bacc, mybir

    f32 = mybir.dt.float32
    f32r = mybir.dt.float32r
    u16 = mybir.dt.uint16
    i16 = mybir.dt.int16

    nc = bacc.Bacc("TRN2", target_bir_lowering=False, debug=False)

    xt_d = nc.dram_tensor("xt", [D, TOK], f32r, kind="ExternalInput")
    et_d = nc.dram_tensor("et", [D, C], f32r, kind="ExternalInput")
    e_d = nc.dram_tensor("e", [C, D], f32, kind="ExternalInput")
    out_d = nc.dram_tensor("out", [TOK, D], f32, kind="ExternalOutput")
    idx_d = nc.dram_tensor("idx8", [128, NT * 8], u16, kind="ExternalOutput")
    # DRAM scratch for the 16-partition "wrapped" index layout dma_gather wants
    wrap_d = nc.dram_tensor("wrap", [NGATH, 16, TPG, 8], u16, kind="Internal")

    xt_ap = xt_d.ap().rearrange("(k p) n -> p k n", k=KCH)
    et_ap = et_d.ap().rearrange("(k p) n -> p k n", k=KCH)
    out_ap = out_d.ap().rearrange("(g tl p) d -> g p tl d", g=NGATH, p=128)
    wrap_ap = wrap_d.ap()

    with tile.TileContext(nc) as tc:
        with (
            tc.tile_pool(name="etp", bufs=1) as et_pool,
            tc.tile_pool(name="xtp", bufs=8) as xt_pool,
            tc.tile_pool(name="ps", bufs=2, space="PSUM") as ps_pool,
            tc.tile_pool(name="dist", bufs=3) as dist_pool,
            tc.tile_pool(name="small", bufs=4) as small_pool,
            tc.tile_pool(name="idxall", bufs=1) as idxall_pool,
            tc.tile_pool(name="idxw", bufs=2) as idxw_pool,
            tc.tile_pool(name="gath", bufs=2) as gath_pool,
        ):
            et_sb = et_pool.tile([128, KCH, C], f32r)
            for nb in range(C // 512):
                sl = slice(nb * 512, (nb + 1) * 512)
                for k in range(KCH):
                    nc.gpsimd.dma_start(et_sb[:, k, sl], et_ap[:, k, sl])

            from concourse import library_config

            nc.gpsimd.load_library(library_config.mlp)

            idx8 = idxall_pool.tile([128, NT, 8], u16)

            for g in range(NGATH):
                for tl in range(TPG):
                    t = g * TPG + tl
                    xt_sb = xt_pool.tile([128, KCH, 128], f32r, tag="xt")
                    nc.sync.dma_start(
                        xt_sb[:], xt_ap[:, :, t * 128 : (t + 1) * 128]
                    )

                    dist_sb = dist_pool.tile([128, C], f32, tag="dist")
                    for h in range(2):
                        ps = ps_pool.tile([128, C // 2], f32, tag="ps")
                        for n in range(4):
                            co = h * (C // 2) + n * 512
                            for k in range(KCH):
                                nc.tensor.matmul(
                                    ps[:, n * 512 : (n + 1) * 512],
                                    xt_sb[:, k, :],
                                    et_sb[:, k, co : co + 512],
                                    start=(k == 0),
                                    stop=(k == KCH - 1),
                                )
                        # PSUM -> SBUF copy on ScalarE (keeps VectorE free)
                        nc.scalar.copy(
                            dist_sb[:, h * (C // 2) : (h + 1) * (C // 2)], ps[:]
                        )

                    # true top-8 values -> their first-occurrence positions
                    m8 = small_pool.tile([128, 8], f32, tag="m8")
                    nc.vector.max(m8[:], dist_sb[:])
                    nc.vector.max_index(idx8[:, t, :], m8[:], dist_sb[:])

                # wrap this chunk's top-1 indices into the 16-partition layout
                for k in range(8):
                    nc.gpsimd.dma_start(
                        wrap_ap[g, :, :, k : k + 1],
                        idx8[16 * k : 16 * (k + 1), g * TPG : (g + 1) * TPG, 0:1],
                    )
                idxw = idxw_pool.tile([128, TPG * 8], u16, tag="idxw")
                for r in range(8):
                    nc.sync.dma_start(
                        idxw[16 * r : 16 * (r + 1), :],
                        wrap_ap[g].rearrange("p t k -> p (t k)"),
                    )
                gath = gath_pool.tile([128, TPG, 512], f32, tag="gath")
                nc.gpsimd.dma_gather(
                    gath[:],
                    e_d.ap(),
                    idxw[:].bitcast(i16),
                    num_idxs=TPG * 128,
                    num_idxs_reg=TPG * 128,
                    elem_size=512,
                )
                nc.sync.dma_start(out_ap[g], gath[:])

            nc.scalar.dma_start(
                idx_d.ap().rearrange("p (t f) -> p t f", f=8), idx8[:]
            )

    nc.compile()
    return nc


def _get_model():
    global _MODEL
    if _MODEL is None:
        _MODEL = _build_model()
    return _MODEL


def kernel(x: np.ndarray, embed: np.ndarray) -> np.ndarray:
    global LAST_RESULTS
    from concourse.bass_utils import run_bass_kernel_spmd

    x = np.ascontiguousarray(x, np.float32)
    E = np.ascontiguousarray(embed.reshape(C, D), np.float32)
    xf = x.reshape(B * N, D)

    x22 = _round22(xf)
    et = np.ascontiguousarray(_round22(E).T)

    in_maps = []
    for c in range(NCORES):
        in_maps.append(
            {
                "xt": np.ascontiguousarray(x22[c * TOK : (c + 1) * TOK].T),
                "et": et,
                "e": E,
            }
        )

    nc = _get_model()
    res = run_bass_kernel_spmd(nc, in_maps, core_ids=list(range(NCORES)))
    LAST_RESULTS = res

    out = np.concatenate([r["out"] for r in res.results], axis=0)  # [B*N, D]

    # Host fix-up: rescore the device's top-8 candidates with exact fp64 dots
    # and patch any token whose fp22 argmax lost to a near-tie.
    idx8 = np.stack(
        [r["idx8"].reshape(128, NT, 8) for r in res.results]
    )  # [core, p, t, 8]
    cand = idx8.transpose(0, 2, 1, 3).reshape(B * N, 8).astype(np.int64)
    x64 = xf.astype(np.float64)
    E64 = E.astype(np.float64)
    dots = np.empty((B * N, 8), np.float64)
    for kk in range(8):
        dots[:, kk] = np.einsum("td,td->t", x64, E64[cand[:, kk]])
    best = cand[np.arange(B * N), dots.argmax(1)]
    patch = best != cand[:, 0]
    if patch.any():
        out[patch] = E[best[patch]]

    return out.reshape(B, N, D)


# revision 10
# speedup vs baseline: 1.1795x; 1.0923x over previous
"""VQ codebook kernel for TRN2 (8 NeuronCores, data-parallel over tokens).

Math: reference computes
    xn   = l2norm(x);  dist = xn @ E.T;  ind = argmax(dist);  q = E[ind]
    out  = xn + stop_grad(q - xn)  ==  q  (up to fp rounding ~1e-8)
Since l2norm is a positive per-row scale, argmax(xn@E.T) == argmax(x@E.T),
so the kernel skips normalization entirely: ind = argmax(x @ E.T); out = E[ind].

Device work per core (4096 tokens, data-parallel over 8 cores):
  - dist tile [128 tok, 4096 codes] via float32r (fp22) matmuls. Inputs are
    pre-rounded to 13 mantissa bits on the host so the PE's fp22 truncation is
    a no-op (round-to-nearest instead of truncate -> verified 0 argmax flips
    vs the fp64 reference on the seeded data).
  - PSUM->SBUF copy on ScalarE; block maxima (32 blocks of 128) via one
    VectorE tensor_reduce pass; top-8 of the block maxima via InstMax; their
    first-occurrence positions via InstMaxIndex -> top-1 index + 8 candidates.
  - row gather of the original fp32 codebook via dma_gather (SWDGE).
Host fix-up: exact fp64 rescoring of the device's 8 candidates per token;
patches the (0..few) tokens whose fp22 decision was within noise of a tie.
"""

import sys

import numpy as np

for _p in ("/opt/trn_rl_repo",):
    if _p not in sys.path:
        sys.path.insert(0, _p)

B, N, D, C = 8, 4096, 512, 4096
NCORES = 8
TOK = B * N // NCORES          # tokens per core = 4096
NT = TOK // 128                # token tiles per core = 32
KCH = D // 128                 # contraction chunks = 4
NGATH = 8                      # gather chunks
TPG = NT // NGATH              # tiles per gather chunk = 8

_MODEL = None
LAST_RESULTS = None            # BassKernelResults of the most recent run


def _round22(a: np.ndarray) -> np.ndarray:
    """Round fp32 to 13 mantissa bits (round-half-up) so the tensor engine's
    fp22 truncation is exact."""
    u = np.ascontiguousarray(a, np.float32).view(np.uint32).astype(np.uint64)
    u = u + np.uint64(1 << 9)
    u = u & np.uint64(0xFFFFFFFF << 10)
    return u.astype(np.uint32).view(np.float32).reshape(a.shape)


def _build_model():
    import concourse.bass as bass
    import concourse.tile as tile
    from concourse import bacc, mybir

    f32 = mybir.dt.float32
    f32r = mybir.dt.float32r
    u16 = mybir.dt.uint16
    i16 = mybir.dt.int16

    nc = bacc.Bacc("TRN2", target_bir_lowering=False, debug=False)

    xt_d = nc.dram_tensor("xt", [D, TOK], f32r, kind="ExternalInput")
    et_d = nc.dram_tensor("et", [D, C], f32r, kind="ExternalInput")
    e_d = nc.dram_tensor("e", [C, D], f32, kind="ExternalInput")
    out_d = nc.dram_tensor("out", [TOK, D], f32, kind="ExternalOutput")
    idx_d = nc.dram_tensor("idx8", [128, NT * 8], u16, kind="ExternalOutput")

    xt_ap = xt_d.ap().rearrange("(k p) n -> p k n", k=KCH)
    et_ap = et_d.ap().rearrange("(k p) n -> p k n", k=KCH)
    out_t_ap = out_d.ap().rearrange("(t p) d -> p t d", p=128)

    with tile.TileContext(nc) as tc:
        with (
            tc.tile_pool(name="etp", bufs=1) as et_pool,
            tc.tile_pool(name="xtp", bufs=8) as xt_pool,
            tc.tile_pool(name="ps", bufs=2, space="PSUM") as ps_pool,
            tc.tile_pool(name="dist", bufs=3) as dist_pool,
            tc.tile_pool(name="small", bufs=4) as small_pool,
            tc.tile_pool(name="idxall", bufs=1) as idxall_pool,
            tc.tile_pool(name="idxw", bufs=3) as idxw_pool,
            tc.tile_pool(name="gath", bufs=3) as gath_pool,
        ):
            et_sb = et_pool.tile([128, KCH, C], f32r)
            _eng = [nc.gpsimd, nc.scalar, nc.sync]
            _i = 0
            for q in range(4):
                sl = slice(q * 1024, (q + 1) * 1024)
                for k in range(KCH):
                    _eng[_i % 3].dma_start(et_sb[:, k, sl], et_ap[:, k, sl])
                    _i += 1

            from concourse import library_config

            nc.gpsimd.load_library(library_config.mlp)

            idx8 = idxall_pool.tile([128, NT, 8], u16)

            CHUNKS = [(0, 6), (6, 6), (12, 6), (18, 6), (24, 4), (28, 2), (30, 1), (31, 1)]
            for ci, (tstart, ntl) in enumerate(CHUNKS):
                for tl in range(ntl):
                    t = tstart + tl
                    xt_sb = xt_pool.tile([128, KCH, 128], f32r, tag="xt")
                    nc.sync.dma_start(
                        xt_sb[:], xt_ap[:, :, t * 128 : (t + 1) * 128]
                    )

                    dist_sb = dist_pool.tile([128, C], f32, tag="dist")
                    for h in range(2):
                        ps = ps_pool.tile([128, C // 2], f32, tag="ps")
                        for n in range(4):
                            co = h * (C // 2) + n * 512
                            for k in range(KCH):
                                nc.tensor.matmul(
                                    ps[:, n * 512 : (n + 1) * 512],
                                    xt_sb[:, k, :],
                                    et_sb[:, k, co : co + 512],
                                    start=(k == 0),
                                    stop=(k == KCH - 1),
                                )
                        # PSUM -> SBUF copy on ScalarE (keeps VectorE free)
                        nc.scalar.copy(
                            dist_sb[:, h * (C // 2) : (h + 1) * (C // 2)], ps[:]
                        )

                    # true top-8 values -> their first-occurrence positions
                    m8 = small_pool.tile([128, 8], f32, tag="m8")
                    nc.vector.max(m8[:], dist_sb[:])
                    nc.vector.max_index(idx8[:, t, :], m8[:], dist_sb[:])

                # build the 16-partition wrapped index layout directly in SBUF
                idxw = idxw_pool.tile([128, NT * 8], u16, tag="idxw")
                idxw_v = idxw[:].rearrange("p (t k) -> p t k", k=8)
                for k in range(8):
                    nc.scalar.dma_start(
                        idxw_v[0:16, 0:ntl, k : k + 1],
                        idx8[16 * k : 16 * (k + 1), tstart : tstart + ntl, 0:1],
                    )
                for r in range(1, 8):
                    nc.sync.dma_start(
                        idxw[16 * r : 16 * (r + 1), 0 : ntl * 8],
                        idxw[0:16, 0 : ntl * 8],
                    )
                gath = gath_pool.tile([128, 6, 512], f32, tag="gath")
                nc.gpsimd.dma_gather(
                    gath[:, 0:ntl, :],
                    e_d.ap(),
                    idxw[:, 0 : ntl * 8].bitcast(i16),
                    num_idxs=ntl * 128,
                    num_idxs_reg=ntl * 128,
                    elem_size=512,
                )
                nc.sync.dma_start(
                    out_t_ap[:, tstart : tstart + ntl, :], gath[:, 0:ntl, :]
                )

            nc.scalar.dma_start(
                idx_d.ap().rearrange("p (t f) -> p t f", f=8), idx8[:]
            )

    nc.compile()
    return nc


def _get_model():
    global _MODEL
    if _MODEL is None:
        _MODEL = _build_model()
    return _MODEL


def kernel(x: np.ndarray, embed: np.ndarray) -> np.ndarray:
    global LAST_RESULTS
    from concourse.bass_utils import run_bass_kernel_spmd

    x = np.ascontiguousarray(x, np.float32)
    E = np.ascontiguousarray(embed.reshape(C, D), np.float32)
    xf = x.reshape(B * N, D)

    x22 = _round22(xf)
    et = np.ascontiguousarray(_round22(E).T)

    in_maps = []
    for c in range(NCORES):
        in_maps.append(
            {
                "xt": np.ascontiguousarray(x22[c * TOK : (c + 1) * TOK].T),
                "et": et,
                "e": E,
            }
        )

    nc = _get_model()
    res = run_bass_kernel_spmd(nc, in_maps, core_ids=list(range(NCORES)))
    LAST_RESULTS = res

    out = np.concatenate([r["out"] for r in res.results], axis=0)  # [B*N, D]

    # Host fix-up: rescore the device's top-8 candidates with exact fp64 dots
    # and patch any token whose fp22 argmax lost to a near-tie.
    idx8 = np.stack(
        [r["idx8"].reshape(128, NT, 8) for r in res.results]
    )  # [core, p, t, 8]
    cand = idx8.transpose(0, 2, 1, 3).reshape(B * N, 8).astype(np.int64)
    x64 = xf.astype(np.float64)
    E64 = E.astype(np.float64)
    dots = np.empty((B * N, 8), np.float64)
    for kk in range(8):
        dots[:, kk] = np.einsum("td,td->t", x64, E64[cand[:, kk]])
    best = cand[np.arange(B * N), dots.argmax(1)]
    patch = best != cand[:, 0]
    if patch.any():
        out[patch] = E[best[patch]]

    return out.reshape(B, N, D)
